# revision 15
# baseline (speedup 1.0000x reference)
"""DeepAR LSTM forward on 8 Trainium2 NeuronCores.

Strategy (data-parallel over batch):
  - B=1024 split as 128 batch elements per core; the L=512 time recurrence
    runs locally on each core.
  - Cell state is batch-major on chip (batch on partitions, gates on the
    free dim) so every elementwise operand pair shares partition base 0,
    which the walrus verifier requires for SBUF-SBUF TensorTensor ops.
  - ONE matmul per step computes all gate pre-activations: the stationary
    operand is a combined [112, 128] column of a chunk tile holding
    [x_t; 1; 0s; v_{t-1}; 0s; o_{t-1}], the moving operand is a combined
    weight matrix.  The gate bias rides the ones-row.  float32r makes the
    matmul single-pass (vs fp32's LOW+HIGH double pass).
  - tanh is folded into sigmoid: tanh(z) = 2*sigmoid(2z) - 1 (g columns
    pre-scaled by 2), so one sigmoid covers i, f, w=sigma(2g).
  - h = o*tanh(c) is never materialized: with v = sigmoid(2c)*o we have
    h = 2v - o, folded into the recurrent/head weights.  o is transposed
    to feature-major EARLY (right after its sigmoid, off the serial
    chain); after u = sigmoid(2c) is transposed, v is computed directly
    in feature-major form into the next step's stationary slot.
  - Heads are weight-stationary bulk matmuls over 4 steps of (v, o)
    feature-major slots; outputs stay feature-major, host transposes.
"""

import sys
from contextlib import ExitStack

import numpy as np

sys.path.insert(0, "/opt/trn_rl_repo")

import concourse.bacc as bacc  # noqa: E402
import concourse.bass as bass  # noqa: E402
import concourse.mybir as mybir  # noqa: E402
import concourse.tile as tile  # noqa: E402

F32 = mybir.dt.float32
F32R = mybir.dt.float32r
AF = mybir.ActivationFunctionType
ALU = mybir.AluOpType

L, B, IN, H, OBS = 512, 1024, 32, 16, 16
NCORES = 8
B_LOC = B // NCORES          # 128 batch rows per core
T_C = 64                     # timesteps per SBUF chunk
HEAD_G = 4                   # timesteps per head matmul

# gate column order in the [*, 64] gate tensors
GI, GF, GW, GO = 0, 16, 32, 48
# combined stationary rows: x 0:32, ones 32, zeros 33:64, o 64:80,
# zeros 80:96, v 96:112  (o sits at 64 so its transpose-out base is legal)
RV, RO, R1 = 96, 64, 112


def build_nc(steps: int = L, b_loc: int = B_LOC):
    """Emit the per-core Bass program (identical on all cores)."""
    nc = bacc.Bacc(None, target_bir_lowering=False)

    xT = nc.dram_tensor("xT", [IN + 1, steps * b_loc], F32R, kind="ExternalInput")
    w_all = nc.dram_tensor("w_all", [R1, 64], F32R, kind="ExternalInput")
    w_h = nc.dram_tensor("w_h", [48, 2 * OBS], F32R, kind="ExternalInput")
    b_h = nc.dram_tensor("b_h", [2 * OBS, 1], F32, kind="ExternalInput")
    id_in = nc.dram_tensor("id_in", [128, 128], F32R, kind="ExternalInput")

    outT = nc.dram_tensor("outT", [2 * OBS, steps * b_loc], F32,
                          kind="ExternalOutput")
    h_last = nc.dram_tensor("h_last", [b_loc, H], F32, kind="ExternalOutput")
    c_last = nc.dram_tensor("c_last", [b_loc, H], F32, kind="ExternalOutput")

    n_chunks = (steps + T_C - 1) // T_C
    assert steps % T_C == 0 and T_C % HEAD_G == 0

    with tile.TileContext(nc) as tc, ExitStack() as ctx:
        singles = ctx.enter_context(tc.tile_pool(name="singles", bufs=1))
        xpool = ctx.enter_context(tc.tile_pool(name="xpool", bufs=3))
        opool = ctx.enter_context(tc.tile_pool(name="opool", bufs=2))
        spool = ctx.enter_context(tc.tile_pool(name="spool", bufs=3))
        small = ctx.enter_context(tc.tile_pool(name="small", bufs=3))
        psum_g = ctx.enter_context(tc.tile_pool(name="psum_g", bufs=2, space="PSUM"))
        psum_u = ctx.enter_context(tc.tile_pool(name="psum_u", bufs=2, space="PSUM"))
        psum_o = ctx.enter_context(tc.tile_pool(name="psum_o", bufs=2, space="PSUM"))
        psum_h = ctx.enter_context(tc.tile_pool(name="psum_h", bufs=2, space="PSUM"))

        # --- constants ---
        wall = singles.tile([R1, 64], F32R)
        # head weights at partitions 64:112 so the head matmul's operands
        # share base partition 64 (hardware requirement); rows 80:96 zero
        wh_t = singles.tile([R1, 2 * OBS], F32R)
        wh = wh_t[RO:R1, :]
        bh = singles.tile([2 * OBS, 1], F32)
        identr = singles.tile([128, 128], F32R)
        nc.sync.dma_start(out=identr, in_=id_in[:])
        nc.sync.dma_start(out=wall, in_=w_all[:])
        nc.sync.dma_start(out=wh, in_=w_h[:])
        nc.sync.dma_start(out=bh, in_=b_h[:])

        cA = singles.tile([b_loc, H], F32)
        cB = singles.tile([b_loc, H], F32)
        nc.vector.memset(cA, 0.0)
        c_tiles = (cA, cB)

        # trailing stationary slot for step L-1's (v, o) -> step-511 heads
        xlast = singles.tile([R1, b_loc], F32R)
        nc.gpsimd.memset(xlast[64:96, :].bitcast(F32), 0.0)

        def new_chunk(ck):
            """Allocate chunk ck's stationary tile, queue DMA + memsets."""
            t0 = ck * T_C
            xc = xpool.tile([R1, T_C * b_loc], F32R)
            # zero bands (DMA rewrites the ones-row 32 afterwards); v slots
            # 64:80 are overwritten per step, 80:96 stays zero for heads
            nc.gpsimd.memset(xc[32:64, :].bitcast(F32), 0.0)
            nc.gpsimd.memset(xc[64:96, :].bitcast(F32), 0.0)
            # v band slot 0 is written by the previous chunk's last step;
            # later slots by this chunk's steps
            nc.sync.dma_start(
                out=xc[: IN + 1, :],
                in_=xT[:, t0 * b_loc : (t0 + T_C) * b_loc],
            )
            if ck == 0:
                nc.gpsimd.memset(xc[RV:R1, :b_loc].bitcast(F32), 0.0)
            return xc

        xc_cur = new_chunk(0)
        xc_next = new_chunk(1) if n_chunks > 1 else None
        last_u = None
        last_ob = None

        for ck in range(n_chunks):
            t0 = ck * T_C
            oc = opool.tile([2 * OBS, T_C * b_loc], F32)

            for tl in range(T_C):
                t = t0 + tl
                sl = slice(tl * b_loc, (tl + 1) * b_loc)
                if tl + 1 < T_C:
                    nsl = slice((tl + 1) * b_loc, (tl + 2) * b_loc)
                    xc_dst = xc_cur
                elif xc_next is not None:
                    nsl = slice(0, b_loc)
                    xc_dst = xc_next
                else:
                    nsl = slice(0, b_loc)
                    xc_dst = xlast

                pg = psum_g.tile([b_loc, 64], F32)
                nc.tensor.matmul(pg, xc_cur[0:R1, sl], wall,
                                 start=True, stop=True)

                s = spool.tile([b_loc, 48], F32)
                ob = small.tile([b_loc, H], F32R)
                nc.scalar.activation(out=s, in_=pg[:, 0:48], func=AF.Sigmoid)
                nc.scalar.activation(out=ob, in_=pg[:, 48:64], func=AF.Sigmoid)

                c_prev = c_tiles[t % 2]
                c_new = c_tiles[(t + 1) % 2]
                wt = small.tile([b_loc, H], F32)
                t1m = small.tile([b_loc, H], F32)
                m1 = small.tile([b_loc, H], F32)
                nc.vector.tensor_scalar(
                    out=wt, in0=s[:, GW : GW + H], scalar1=2.0, scalar2=-1.0,
                    op0=ALU.mult, op1=ALU.add,
                )
                nc.vector.tensor_tensor(t1m, s[:, GI : GI + H], wt, op=ALU.mult)
                nc.vector.tensor_tensor(m1, s[:, GF : GF + H], c_prev,
                                        op=ALU.mult)
                nc.vector.tensor_tensor(c_new, m1, t1m, op=ALU.add)

                # o -> feature-major, off the serial chain (transpose
                # outputs must land at PSUM base 0; the copy shifts to the
                # o band at partition 64).  Emitted here so the DVE copy
                # queues after c_new and drains during the u sigmoid.
                po = psum_o.tile([H, b_loc], F32R)
                nc.tensor.transpose(po[:], ob[:], identr[:])
                nc.vector.tensor_copy(xc_dst[RO : RO + H, nsl], po[:])

                u = small.tile([b_loc, H], F32R)
                nc.scalar.activation(out=u, in_=c_new, func=AF.Sigmoid, scale=2.0)

                # u -> feature-major, then v = u*o directly feature-major
                pu = psum_u.tile([H, b_loc], F32R)
                nc.tensor.transpose(pu[:], u[:], identr[:])
                nc.vector.tensor_tensor(
                    xc_dst[RV : RV + H, nsl], pu[:], xc_dst[RO : RO + H, nsl],
                    op=ALU.mult,
                )
                last_u, last_ob = u, ob

                # heads over completed 4-slot windows [4k, 4k+4)
                if tl % HEAD_G == HEAD_G - 2:
                    w0 = tl + 2 - HEAD_G
                    wsl = slice(w0 * b_loc, (w0 + HEAD_G) * b_loc)
                    ph = psum_h.tile([2 * OBS, HEAD_G * b_loc], F32)
                    nc.tensor.matmul(ph, wh, xc_cur[RO:R1, wsl],
                                     start=True, stop=True)
                    nc.scalar.activation(out=oc[:, wsl], in_=ph[:],
                                         func=AF.Identity, bias=bh[:])

            # chunk ck's slots s hold (v,o)_{t0+s-1} -> output steps t0-1..t0+30
            if ck == 0:
                nc.sync.dma_start(
                    out=outT[:, 0 : (T_C - 1) * b_loc],
                    in_=oc[:, b_loc:],
                )
            else:
                nc.sync.dma_start(
                    out=outT[:, (t0 - 1) * b_loc : (t0 + T_C - 1) * b_loc],
                    in_=oc[:],
                )
            xc_cur = xc_next
            xc_next = new_chunk(ck + 2) if ck + 2 < n_chunks else None

        # final step's heads from xlast
        ph = psum_h.tile([2 * OBS, b_loc], F32)
        ol = small.tile([2 * OBS, b_loc], F32)
        nc.tensor.matmul(ph, wh, xlast[RO:R1, :], start=True, stop=True)
        nc.scalar.activation(out=ol, in_=ph[:], func=AF.Identity, bias=bh[:])
        nc.sync.dma_start(out=outT[:, (steps - 1) * b_loc :], in_=ol[:])

        # h = o*(2u - 1) batch-major from the final step's u, o
        th = small.tile([b_loc, H], F32)
        hl = small.tile([b_loc, H], F32)
        nc.vector.tensor_tensor(th, last_u, last_ob, op=ALU.mult)
        nc.vector.scalar_tensor_tensor(
            out=hl, in0=th, scalar=2.0, in1=last_ob,
            op0=ALU.mult, op1=ALU.subtract,
        )
        nc.sync.dma_start(out=h_last[:], in_=hl[:])
        nc.sync.dma_start(out=c_last[:], in_=c_tiles[steps % 2][:])

    return nc


def prep_weights(W_ih, W_hh, b_ih, b_hh, W_mu, b_mu, W_sig, b_sig):
    W = np.asarray(W_ih, np.float32)        # (64, 32)  rows i,f,g,o
    U = np.asarray(W_hh, np.float32)        # (64, 16)
    b = np.asarray(b_ih, np.float32) + np.asarray(b_hh, np.float32)
    Wm = np.asarray(W_mu, np.float32)       # (16, 16)
    Ws = np.asarray(W_sig, np.float32)

    # gate columns [i, f, w(g), o]; g block pre-scaled by 2 for the tanh fold
    w_all = np.zeros((R1, 64), np.float32)
    for bi in range(4):
        r = slice(bi * H, (bi + 1) * H)
        col = slice(bi * H, (bi + 1) * H)
        sc = 2.0 if bi == 2 else 1.0
        w_all[:IN, col] = sc * W[r].T
        w_all[IN, col] = sc * b[r]
        w_all[RV : RV + H, col] = sc * 2.0 * U[r].T
        w_all[RO : RO + H, col] = sc * -1.0 * U[r].T

    # head weight rows match stationary rows 64:112: o, zeros, v
    w_h = np.zeros((48, 2 * OBS), np.float32)
    w_h[0:16, :OBS] = -Wm.T
    w_h[0:16, OBS:] = -Ws.T
    w_h[32:48, :OBS] = 2.0 * Wm.T
    w_h[32:48, OBS:] = 2.0 * Ws.T
    b_h = np.concatenate([np.asarray(b_mu, np.float32),
                          np.asarray(b_sig, np.float32)]).reshape(2 * OBS, 1)
    return w_all, w_h, b_h


def kernel(external_input_seq, W_ih, W_hh, b_ih, b_hh, W_mu, b_mu, W_sig, b_sig,
           _trace=False):
    from concourse.bass_utils import run_bass_kernel_spmd

    x = np.asarray(external_input_seq, np.float32)      # (L, B, IN)
    w_all, w_h, b_h = prep_weights(W_ih, W_hh, b_ih, b_hh, W_mu, b_mu,
                                   W_sig, b_sig)

    nc = build_nc(L, B_LOC)
    nc.compile()
    in_maps = []
    for c in range(NCORES):
        xc = x[:, c * B_LOC : (c + 1) * B_LOC, :]       # (L, B_loc, IN)
        xT = np.empty((IN + 1, L * B_LOC), np.float32)
        xT[:IN] = xc.transpose(2, 0, 1).reshape(IN, L * B_LOC)
        xT[IN] = 1.0
        in_maps.append({"xT": xT, "w_all": w_all, "w_h": w_h, "b_h": b_h,
                        "id_in": np.eye(128, dtype=np.float32)})

    res = run_bass_kernel_spmd(nc, in_maps, list(range(NCORES)), trace=_trace)

    mu = np.empty((L, B, OBS), np.float32)
    sg = np.empty((L, B, OBS), np.float32)
    hl = np.empty((1, B, H), np.float32)
    cl = np.empty((1, B, H), np.float32)
    for c in range(NCORES):
        r = res.results[c]
        bs = slice(c * B_LOC, (c + 1) * B_LOC)
        ot = r["outT"].reshape(2 * OBS, L, B_LOC).transpose(1, 2, 0)
        mu[:, bs, :] = ot[:, :, :OBS]
        sg[:, bs, :] = ot[:, :, OBS:]
        hl[0, bs, :] = r["h_last"]
        cl[0, bs, :] = r["c_last"]

    if _trace:
        kernel.last_exec_time_ns = res.exec_time_ns
    return mu, sg, hl, cl
